# revision 27
# baseline (speedup 1.0000x reference)
"""Sliding-window (banded causal) multi-head attention on 8 TRN2 NeuronCores.

Sharding: 8 cores = 2 batches x 4 head-groups (4 heads of 64 dims each).
Each core computes QKV projections for its 4 heads, RoPE, banded flash
attention (window 1024), and a partial output projection (its 256 columns
of wo). The host sums the 4 partial outputs per batch element.

Device layout (per core) follows the proven v1 design: bf16 operands,
fp32 PSUM, activations pre-transposed on host (xT), Q/K produced
transposed, scores computed as [k, q] blocks feeding the PV matmul
directly, softmax denominators from a ones-column in V, no running-max.

Schedule (v3): the cost model says matmul time is out-cols x 0.4167ns
with the PE ramping to full clock only after 3us of continuous work,
and the exp stream on Activation (~58us total) is the attention-phase
bound. So:
  - a memset-fed warmup accumulation chain keeps the PE busy from
    ~0.5us, so the real projections start fully ramped
  - attention pass A starts right after the m=0 projection + partial V;
    the remaining V tiles, the m=1 QK projection (256-wide half
    chains), and the m=1 RoPE are woven into pass A as PE filler, which
    starts Activation's exp stream ~25us earlier than the v1 phase
    ordering and hides it under PE work
  - pass B runs q-tiles descending so the tail drains through tile 0
    (shortest chain); the output projection + stores trail 3 tiles
  - psum drains stay off the Activation queue (it is exp-bound); output
    stores ride the idle SP queue
"""

from contextlib import ExitStack

import numpy as np
import ml_dtypes

import concourse.bass as bass
import concourse.tile as tile
from concourse import bacc, mybir
from concourse.bass_utils import run_bass_kernel_spmd

BF16 = mybir.dt.bfloat16
F32 = mybir.dt.float32

B, S, H = 2, 2048, 1024
NH, HD = 16, 64
WINDOW = 1024
ROPE_THETA = 10000.0
MAX_POS = 2048
N_CORES = 8
HG = 4                      # heads per core
GD = HG * HD                # 256: head-group dim per core
P = 128
NQT = S // P                # 16 q tiles
WT = WINDOW // P            # 8: window in tiles
CH = H // P                 # 8 contraction chunks
VW = HD + 1                 # 65: V width with ones column
K_WARM = 22                 # warmup matmuls (PE p-state ramp during DMA)

_cache = {}


def _build():
    nc = bacc.Bacc("TRN2", target_bir_lowering=False, debug=False,
                   enable_asserts=False, num_devices=N_CORES)

    xT_d = nc.dram_tensor("xT", [H, S], BF16, kind="ExternalInput")
    wqT_d = nc.dram_tensor("wqT", [H, GD], BF16, kind="ExternalInput")
    wkT_d = nc.dram_tensor("wkT", [H, GD], BF16, kind="ExternalInput")
    wvT_d = nc.dram_tensor("wvT", [H, GD], BF16, kind="ExternalInput")
    woT_d = nc.dram_tensor("woT", [GD, H], BF16, kind="ExternalInput")
    cosT_d = nc.dram_tensor("cosT", [P, S], BF16, kind="ExternalInput")
    sinTs_d = nc.dram_tensor("sinTs", [P, S], BF16, kind="ExternalInput")
    bq_d = nc.dram_tensor("bq2", [P, 2], F32, kind="ExternalInput")
    bk_d = nc.dram_tensor("bk2", [P, 2], F32, kind="ExternalInput")
    bqs_d = nc.dram_tensor("bq2s", [P, 2], F32, kind="ExternalInput")
    bks_d = nc.dram_tensor("bk2s", [P, 2], F32, kind="ExternalInput")
    # combined [diag | far] edge masks, bf16 0/1
    masks_d = nc.dram_tensor("masks", [P, 2 * P], BF16, kind="ExternalInput")
    out_d = nc.dram_tensor("out", [S, H], F32, kind="ExternalOutput")

    with tile.TileContext(nc) as tc, ExitStack() as ctx:
        const = ctx.enter_context(tc.tile_pool(name="const", bufs=1))
        qk = ctx.enter_context(tc.tile_pool(name="qk", bufs=1))
        vp = ctx.enter_context(tc.tile_pool(name="vp", bufs=1))
        pp = ctx.enter_context(tc.tile_pool(name="pp", bufs=6))
        cxp = ctx.enter_context(tc.tile_pool(name="cxp", bufs=1))
        osb = ctx.enter_context(tc.tile_pool(name="osb", bufs=2))
        sm = ctx.enter_context(tc.tile_pool(name="sm", bufs=3))

        # ---- loads: same split/cadence as v1 (proven to keep up with the
        # m=0 projection), masks moved earlier for nothing extra ----
        wq_sb = const.tile([P, CH * GD], BF16, name="wq_sb")
        wk_sb = const.tile([P, CH * GD], BF16, name="wk_sb")
        wv_sb = const.tile([P, CH * GD], BF16, name="wv_sb")
        wo_sb = const.tile([P, 2 * H], BF16, name="wo_sb")
        x_sb = const.tile([P, CH * S], BF16, name="x_sb")
        cosT = const.tile([P, S], BF16, name="cosT")
        sinTs = const.tile([P, S], BF16, name="sinTs")
        bq_sb = const.tile([P, 2], F32, name="bq_sb")
        bk_sb = const.tile([P, 2], F32, name="bk_sb")
        bqs_sb = const.tile([P, 2], F32, name="bqs_sb")
        bks_sb = const.tile([P, 2], F32, name="bks_sb")
        masks = const.tile([P, 2 * P], BF16, name="masks")
        warm_in = const.tile([P, 2 * P], BF16, name="warm_in")

        def chunked(dram, w):
            return dram.ap().rearrange("(c p) w -> p c w", p=P)

        xv = x_sb.rearrange("p (c w) -> p c w", c=CH)
        xs = chunked(xT_d, S)
        # wq/wk split by chunk-halves so the interleaved m=0 projection has
        # both weights for early chunks right as x c0 lands
        wqv = wq_sb.rearrange("p (c w) -> p c w", c=CH)
        wkv = wk_sb.rearrange("p (c w) -> p c w", c=CH)
        wqs = chunked(wqT_d, GD)
        wks = chunked(wkT_d, GD)
        nc.scalar.dma_start(wqv[:, 0:4], wqs[:, 0:4])
        nc.scalar.dma_start(wkv[:, 0:4], wks[:, 0:4])
        for lo, hi in ((0, 1), (1, 2), (2, 3), (3, 4), (4, 6), (6, 8)):
            nc.sync.dma_start(xv[:, lo:hi], xs[:, lo:hi])
        nc.scalar.dma_start(wqv[:, 4:8], wqs[:, 4:8])
        nc.scalar.dma_start(wkv[:, 4:8], wks[:, 4:8])
        nc.scalar.dma_start(wv_sb.rearrange("p (c w) -> p c w", c=CH),
                            chunked(wvT_d, GD))
        nc.scalar.dma_start(cosT[:], cosT_d.ap())
        nc.scalar.dma_start(sinTs[:], sinTs_d.ap())
        nc.scalar.dma_start(wo_sb.rearrange("p (c w) -> p c w", c=2),
                            chunked(woT_d, H))
        nc.scalar.dma_start(bq_sb[:], bq_d.ap())
        nc.scalar.dma_start(bk_sb[:], bk_d.ap())
        nc.scalar.dma_start(bqs_sb[:], bqs_d.ap())
        nc.scalar.dma_start(bks_sb[:], bks_d.ap())
        nc.scalar.dma_start(masks[:], masks_d.ap())

        def xc(c):
            return x_sb[:, c * S:(c + 1) * S]

        def wc(w_sb, c, width=GD):
            return w_sb[:, c * width:(c + 1) * width]

        q_sb = [qk.tile([P, S], BF16, name=f"q{m}") for m in range(2)]
        k_sb = [qk.tile([P, S], BF16, name=f"k{m}") for m in range(2)]
        qs_sb = [qk.tile([P, S], BF16, name=f"qs{m}") for m in range(2)]
        ks_sb = [qk.tile([P, S], BF16, name=f"ks{m}") for m in range(2)]
        v_sb = [vp.tile([P, HG * VW], BF16, name=f"v{t}") for t in range(NQT)]
        ctx_sb = [cxp.tile([P, S], BF16, name=f"cx{m}") for m in range(2)]
        cx_raw = [cxp.tile([VW, 2 * S], BF16, name=f"cxr{m}") for m in range(2)]
        _osb = [osb.tile([P, H], F32, tag="osb", name=f"ot{t}", bufs=4)
                for t in range(NQT)]

        # ones columns for the softmax denominators (Pool, idle early)
        nc.gpsimd.memset(warm_in[:], 0.5)
        for t in range(NQT):
            nc.gpsimd.memset(v_sb[t][:], 1.0)

        def rope_piece(m, lo, hi, src, shf, bc, bs):
            cl = slice(lo, hi)
            for hb in range(2):
                o = hb * HD
                nc.sync.dma_start(shf[m][o:o + 32, cl],
                                  src[m][o + 32:o + 64, cl])
                nc.sync.dma_start(shf[m][o + 32:o + 64, cl],
                                  src[m][o:o + 32, cl])
            nc.vector.scalar_tensor_tensor(
                shf[m][:, cl], shf[m][:, cl], bs[:, m:m + 1], sinTs[:, cl],
                mybir.AluOpType.add, mybir.AluOpType.mult)
            nc.vector.scalar_tensor_tensor(
                src[m][:, cl], src[m][:, cl], bc[:, m:m + 1], cosT[:, cl],
                mybir.AluOpType.add, mybir.AluOpType.mult)
            nc.vector.tensor_add(src[m][:, cl], src[m][:, cl], shf[m][:, cl])

        def attn_scores(sp, mt, qi):
            kt0 = max(0, qi - WT)
            # block order: [diag, far?, middles...]
            kts = [qi]
            n_edge = 1
            if qi >= WT:
                kts.append(kt0)
                n_edge = 2
            kts.extend(range(kt0 + (1 if qi >= WT else 0), qi))

            s_ps = [sp.tile([P, WT * P + P], F32, tag="sp",
                            name=f"sps{mt}{qi}{hb}") for hb in range(2)]
            # interleave the two heads' score matmuls: lhsT base partitions
            # 0/64 give disjoint PE row groups -> concurrent on hardware
            for i, kt in enumerate(kts):
                for hb in range(2):
                    ho = hb * HD
                    nc.tensor.matmul(
                        s_ps[hb][:, i * P:(i + 1) * P],
                        k_sb[mt][ho:ho + HD, kt * P:(kt + 1) * P],
                        q_sb[mt][ho:ho + HD, qi * P:(qi + 1) * P],
                        start=True, stop=True)
            prb = []
            for hb in range(2):
                h = mt * 2 + hb
                nkt = len(kts)
                probs = pp.tile([P, WT * P + P], BF16, tag="pp",
                                name=f"pr{h}{qi}")
                nc.scalar.activation(
                    probs[:, 0:nkt * P], s_ps[hb][:, 0:nkt * P],
                    mybir.ActivationFunctionType.Exp,
                    scale=float(1.0 / np.sqrt(HD)))
                nc.vector.tensor_mul(
                    probs[:, 0:n_edge * P], probs[:, 0:n_edge * P],
                    masks[:, 0:n_edge * P])
                prb.append(probs)
            return kts, n_edge, prb

        def attn_ctx(cp, mt, qi, kts, n_edge, prb):
            nkt = len(kts)
            ctx_ps = cp.tile([VW, 2 * P], F32, tag="ctx", name=f"cps{mt}{qi}")
            for hb in range(2):
                h = mt * 2 + hb
                # ctx^T [65, q]: middles first, masked edge blocks last
                issue = list(range(n_edge, nkt)) + list(range(n_edge))
                for j, i in enumerate(issue):
                    nc.tensor.matmul(
                        ctx_ps[:, hb * P:(hb + 1) * P],
                        v_sb[kts[i]][:, h * VW:(h + 1) * VW],
                        prb[hb][:, i * P:(i + 1) * P],
                        start=(j == 0), stop=(j == nkt - 1))
            # drain PSUM with one unnormalized copy (incl. the sums row)
            nc.vector.tensor_copy(
                cx_raw[mt][:, qi * 2 * P:(qi + 1) * 2 * P], ctx_ps[:])

        def norm2(mt, qi0):
            # normalize q-tiles qi0, qi0+1 (broadcast + recip + 4 muls)
            c0 = qi0 * 2 * P
            rinv = sm.tile([1, 4 * P], F32, tag="rinv", name=f"ri{mt}{qi0}")
            nc.vector.reciprocal(rinv[:], cx_raw[mt][HD:HD + 1, c0:c0 + 4 * P])
            rbc = sm.tile([P, 4 * P], F32, tag="rbc", name=f"rb{mt}{qi0}")
            nc.gpsimd.partition_broadcast(rbc[:], rinv[:])
            for qj in (qi0, qi0 + 1):
                for hb in range(2):
                    ho = hb * HD
                    nc.gpsimd.tensor_mul(
                        ctx_sb[mt][ho:ho + HD, qj * P:(qj + 1) * P],
                        cx_raw[mt][0:HD, qj * 2 * P + hb * P:
                                   qj * 2 * P + (hb + 1) * P],
                        rbc[0:HD, (qj - qi0) * 2 * P + hb * P:
                            (qj - qi0) * 2 * P + (hb + 1) * P])

        def vchain(cp, t):
            ps = cp.tile([P, 512], F32, tag="ctx", name=f"pv{t}")
            for c in range(CH):
                nc.tensor.matmul(ps[:, 0:GD], xc(c)[:, t * P:(t + 1) * P],
                                 wc(wv_sb, c), start=(c == 0),
                                 stop=(c == CH - 1))
            vdst = v_sb[t].rearrange("p (h d) -> p h d", h=HG)[:, :, 0:HD]
            vsrc = ps[:, 0:GD].rearrange("p (h d) -> p h d", h=HG)
            nc.scalar.copy(vdst, vsrc)

        def m1half(cp, w_sb, dest, n, half):
            lo = n * 512 + half * 256
            ps = cp.tile([P, 512], F32, tag="ctx", name=f"pm1{n}{half}")
            for c in range(CH):
                nc.tensor.matmul(
                    ps[:, 0:256], wc(w_sb, c)[:, P:2 * P],
                    xc(c)[:, lo:lo + 256],
                    start=(c == 0), stop=(c == CH - 1))
            nc.vector.tensor_copy(dest[1][:, lo:lo + 256], ps[:, 0:256])

        def outproj_t(cp, t, last=False):
            o_sb = _osb[t]
            for n in range(2):
                ps = cp.tile([P, 512], F32, tag="ctx", name=f"po{t}{n}")
                for c in range(2):
                    nc.tensor.matmul(
                        ps[:], ctx_sb[c][:, t * P:(t + 1) * P],
                        wc(wo_sb, c, H)[:, n * 512:(n + 1) * 512],
                        start=(c == 0), stop=(c == 1))
                nc.vector.tensor_copy(o_sb[:, n * 512:(n + 1) * 512], ps[:])
                if last:
                    # past the final exps: halves go out on both queues
                    e = nc.sync if n == 0 else nc.scalar
                    e.dma_start(out_d.ap()[t * P:(t + 1) * P,
                                           n * 512:(n + 1) * 512],
                                o_sb[:, n * 512:(n + 1) * 512])
            if not last:
                nc.sync.dma_start(out_d.ap()[t * P:(t + 1) * P, :], o_sb[:])

        # ---- phase 1: warmup + QK m=0 projection + rope(0) ----
        with tc.tile_pool(name="pj", bufs=8, space="PSUM") as pj:
            warm = pj.tile([P, 512], F32, tag="pj", name="warm")
            for i in range(K_WARM):
                nc.tensor.matmul(warm[:, 0:2 * P], warm_in[:, 0:P],
                                 warm_in[:], start=(i == 0),
                                 stop=(i == K_WARM - 1))
            pss = [[pj.tile([P, 512], F32, tag="pj", name=f"pj{i}{n}")
                    for n in range(4)] for i in range(2)]
            for c in range(CH):
                for i, w in enumerate((wq_sb, wk_sb)):
                    for n in range(4):
                        nc.tensor.matmul(
                            pss[i][n][:], wc(w, c)[:, 0:P],
                            xc(c)[:, n * 512:(n + 1) * 512],
                            start=(c == 0), stop=(c == CH - 1))
            # drain + rope per quarter so attention can start after quarter 0
            for n in range(4):
                for i, d in enumerate((q_sb, k_sb)):
                    nc.scalar.copy(d[0][:, n * 512:(n + 1) * 512],
                                   pss[i][n][:])
                rope_piece(0, n * 512, (n + 1) * 512,
                           q_sb, qs_sb, bq_sb, bqs_sb)
                rope_piece(0, n * 512, (n + 1) * 512,
                           k_sb, ks_sb, bk_sb, bks_sb)

        # ---- main phase: attention A (+fillers), attention B (+outproj) ----
        with tc.tile_pool(name="sp", bufs=2, space="PSUM") as sp, \
             tc.tile_pool(name="cp", bufs=2, space="PSUM") as cp:
            # m=1 half-chain order feeds rope(1) quarter by quarter
            m1args = [(wq_sb, q_sb, 0), (wq_sb, q_sb, 1),
                      (wk_sb, k_sb, 0), (wk_sb, k_sb, 1),
                      (wq_sb, q_sb, 2), (wq_sb, q_sb, 3),
                      (wk_sb, k_sb, 2), (wk_sb, k_sb, 3)]

            def m1_step(j):
                w_sb_, dest_, n_ = m1args[j]
                m1half(cp, w_sb_, dest_, n_, 0)
                m1half(cp, w_sb_, dest_, n_, 1)
                shf_ = qs_sb if dest_ is q_sb else ks_sb
                bc_ = bq_sb if dest_ is q_sb else bk_sb
                bs_ = bqs_sb if dest_ is q_sb else bks_sb
                rope_piece(1, n_ * 512, (n_ + 1) * 512,
                           dest_, shf_, bc_, bs_)

            # fillers before pass A: V tiles 0-3, first m=1 quarter
            for t in range(4):
                vchain(cp, t)
            m1_step(0)

            # ---- pass A (mt=0), ascending ----
            pend = None
            for qi in range(NQT):
                cur = attn_scores(sp, 0, qi)
                if pend is not None:
                    attn_ctx(cp, 0, qi - 1, *pend)
                    if qi % 2 == 0 and qi >= 2:
                        norm2(0, qi - 2)
                pend = cur
                # PE filler work, after the critical attention ops
                if qi + 4 < NQT:
                    vchain(cp, qi + 4)
                if qi <= 6:
                    m1_step(qi + 1)
            attn_ctx(cp, 0, NQT - 1, *pend)
            norm2(0, NQT - 2)

            # ---- pass B (mt=1), descending; outproj trails 3 ----
            pend = None
            for qi in range(NQT - 1, -1, -1):
                cur = attn_scores(sp, 1, qi)
                if pend is not None:
                    attn_ctx(cp, 1, qi + 1, *pend)
                    if qi % 2 == 1:
                        norm2(1, qi + 1)
                if qi <= 12:
                    outproj_t(cp, qi + 3)
                pend = cur
            attn_ctx(cp, 1, 0, *pend)
            norm2(1, 0)
            outproj_t(cp, 2)
            outproj_t(cp, 1, last=True)
            outproj_t(cp, 0, last=True)

    nc.compile()
    return nc


def _rope_tables():
    inv_freq = 1.0 / (ROPE_THETA ** (np.arange(0, HD, 2, dtype=np.float64) / HD))
    t = np.arange(MAX_POS, dtype=np.float64)
    freqs = np.outer(t, inv_freq)                       # [MAX_POS, 32]
    emb = np.concatenate([freqs, freqs], axis=-1)       # [MAX_POS, 64]
    return np.cos(emb).astype(np.float32), np.sin(emb).astype(np.float32)


def kernel(hidden_states, position_ids, wq, bq, wk, bk, wv, bv, wo, bo):
    bf16 = ml_dtypes.bfloat16
    if "nc" not in _cache:
        _cache["nc"] = _build()
    nc = _cache["nc"]

    cos_t, sin_t = _rope_tables()
    pos = np.clip(np.asarray(position_ids), 0, MAX_POS - 1).astype(np.int64)

    maskd = np.triu(np.ones((P, P), np.float32))        # k <= q (diag block)
    maskf = np.tril(np.ones((P, P), np.float32), -1)    # k > q  (far block)
    masks = np.concatenate([maskd, maskf], axis=1).astype(bf16)

    in_maps = []
    for core in range(N_CORES):
        b, g = core // HG, core % HG
        sl = slice(g * GD, (g + 1) * GD)
        cos_b = cos_t[pos[b]]                            # [S, 64]
        sin_b = sin_t[pos[b]]
        cosT = np.tile(cos_b.T, (2, 1)).astype(bf16)     # [128, S]
        sin_sgn = sin_b.T.copy()                         # [64, S]
        sin_sgn[0:32] *= -1.0
        sinTs = np.tile(sin_sgn, (2, 1)).astype(bf16)
        in_maps.append({
            "xT": np.ascontiguousarray(hidden_states[b].T).astype(bf16),
            "wqT": np.ascontiguousarray(wq[sl].T).astype(bf16),
            "wkT": np.ascontiguousarray(wk[sl].T).astype(bf16),
            "wvT": np.ascontiguousarray(wv[sl].T).astype(bf16),
            "woT": np.ascontiguousarray(wo[:, sl].T).astype(bf16),
            "cosT": cosT,
            "sinTs": sinTs,
            "bq2": np.ascontiguousarray(
                bq[sl].reshape(2, P).T).astype(np.float32),
            "bk2": np.ascontiguousarray(
                bk[sl].reshape(2, P).T).astype(np.float32),
            "bq2s": np.ascontiguousarray(
                bq[sl].reshape(2, 2, 2, 32)[:, :, ::-1].reshape(
                    2, P).T).astype(np.float32),
            "bk2s": np.ascontiguousarray(
                bk[sl].reshape(2, 2, 2, 32)[:, :, ::-1].reshape(
                    2, P).T).astype(np.float32),
            "masks": masks,
        })

    res = run_bass_kernel_spmd(nc, in_maps, core_ids=list(range(N_CORES)))

    const_off = (wo @ bv + bo).astype(np.float32)        # host-folded biases
    out = np.empty((B, S, H), dtype=np.float32)
    for b in range(B):
        acc = res.results[b * HG]["out"].astype(np.float32).copy()
        for g in range(1, HG):
            acc += res.results[b * HG + g]["out"]
        out[b] = acc + const_off[None, :]
    return out


# revision 30
# speedup vs baseline: 1.0716x; 1.0716x over previous
"""Sliding-window (banded causal) multi-head attention on 8 TRN2 NeuronCores.

Sharding: 8 cores = 2 batches x 4 head-groups (4 heads of 64 dims each).
Each core computes QKV projections for its 4 heads, RoPE, banded flash
attention (window 1024), and a partial output projection (its 256 columns
of wo). The host sums the 4 partial outputs per batch element.

Device layout (per core) follows the proven v1 design: bf16 operands,
fp32 PSUM, activations pre-transposed on host (xT), Q/K produced
transposed, scores computed as [k, q] blocks feeding the PV matmul
directly, softmax denominators from a ones-column in V, no running-max.

Schedule (v3): the cost model says matmul time is out-cols x 0.4167ns
with the PE ramping to full clock only after 3us of continuous work,
and the exp stream on Activation (~58us total) is the attention-phase
bound. So:
  - a memset-fed warmup accumulation chain keeps the PE busy from
    ~0.5us, so the real projections start fully ramped
  - attention pass A starts right after the m=0 projection + partial V;
    the remaining V tiles, the m=1 QK projection (256-wide half
    chains), and the m=1 RoPE are woven into pass A as PE filler, which
    starts Activation's exp stream ~25us earlier than the v1 phase
    ordering and hides it under PE work
  - pass B runs q-tiles descending so the tail drains through tile 0
    (shortest chain); the output projection + stores trail 3 tiles
  - psum drains stay off the Activation queue (it is exp-bound); output
    stores ride the idle SP queue
"""

from contextlib import ExitStack

import numpy as np
import ml_dtypes

import concourse.bass as bass
import concourse.tile as tile
from concourse import bacc, mybir
from concourse.bass_utils import run_bass_kernel_spmd

BF16 = mybir.dt.bfloat16
F32 = mybir.dt.float32

B, S, H = 2, 2048, 1024
NH, HD = 16, 64
WINDOW = 1024
ROPE_THETA = 10000.0
MAX_POS = 2048
N_CORES = 8
HG = 4                      # heads per core
GD = HG * HD                # 256: head-group dim per core
P = 128
NQT = S // P                # 16 q tiles
WT = WINDOW // P            # 8: window in tiles
CH = H // P                 # 8 contraction chunks
VW = HD + 1                 # 65: V width with ones column
K_WARM = 18                 # warmup matmuls (PE p-state ramp during DMA)

_cache = {}


def _build():
    nc = bacc.Bacc("TRN2", target_bir_lowering=False, debug=False,
                   enable_asserts=False, num_devices=N_CORES)

    xT_d = nc.dram_tensor("xT", [H, S], BF16, kind="ExternalInput")
    wqT_d = nc.dram_tensor("wqT", [H, GD], BF16, kind="ExternalInput")
    wkT_d = nc.dram_tensor("wkT", [H, GD], BF16, kind="ExternalInput")
    wvT_d = nc.dram_tensor("wvT", [H, GD], BF16, kind="ExternalInput")
    woT_d = nc.dram_tensor("woT", [GD, H], BF16, kind="ExternalInput")
    cosT_d = nc.dram_tensor("cosT", [P, S], BF16, kind="ExternalInput")
    sinTs_d = nc.dram_tensor("sinTs", [P, S], BF16, kind="ExternalInput")
    bq_d = nc.dram_tensor("bq2", [P, 2], F32, kind="ExternalInput")
    bk_d = nc.dram_tensor("bk2", [P, 2], F32, kind="ExternalInput")
    bqs_d = nc.dram_tensor("bq2s", [P, 2], F32, kind="ExternalInput")
    bks_d = nc.dram_tensor("bk2s", [P, 2], F32, kind="ExternalInput")
    # combined [diag | far] edge masks, bf16 0/1
    masks_d = nc.dram_tensor("masks", [P, 2 * P], BF16, kind="ExternalInput")
    out_d = nc.dram_tensor("out", [S, H], F32, kind="ExternalOutput")

    with tile.TileContext(nc) as tc, ExitStack() as ctx:
        const = ctx.enter_context(tc.tile_pool(name="const", bufs=1))
        qk = ctx.enter_context(tc.tile_pool(name="qk", bufs=1))
        vp = ctx.enter_context(tc.tile_pool(name="vp", bufs=1))
        pp = ctx.enter_context(tc.tile_pool(name="pp", bufs=6))
        cxp = ctx.enter_context(tc.tile_pool(name="cxp", bufs=1))
        osb = ctx.enter_context(tc.tile_pool(name="osb", bufs=2))
        sm = ctx.enter_context(tc.tile_pool(name="sm", bufs=3))

        # ---- loads: same split/cadence as v1 (proven to keep up with the
        # m=0 projection), masks moved earlier for nothing extra ----
        wq_sb = const.tile([P, CH * GD], BF16, name="wq_sb")
        wk_sb = const.tile([P, CH * GD], BF16, name="wk_sb")
        wv_sb = const.tile([P, CH * GD], BF16, name="wv_sb")
        wo_sb = const.tile([P, 2 * H], BF16, name="wo_sb")
        x_sb = const.tile([P, CH * S], BF16, name="x_sb")
        cosT = const.tile([P, S], BF16, name="cosT")
        sinTs = const.tile([P, S], BF16, name="sinTs")
        bq_sb = const.tile([P, 2], F32, name="bq_sb")
        bk_sb = const.tile([P, 2], F32, name="bk_sb")
        bqs_sb = const.tile([P, 2], F32, name="bqs_sb")
        bks_sb = const.tile([P, 2], F32, name="bks_sb")
        masks = const.tile([P, 2 * P], BF16, name="masks")
        warm_in = const.tile([P, 2 * P], BF16, name="warm_in")

        def chunked(dram, w):
            return dram.ap().rearrange("(c p) w -> p c w", p=P)

        xv = x_sb.rearrange("p (c w) -> p c w", c=CH)
        xs = chunked(xT_d, S)
        # wq/wk split by chunk-halves so the interleaved m=0 projection has
        # both weights for early chunks right as x c0 lands
        wqv = wq_sb.rearrange("p (c w) -> p c w", c=CH)
        wkv = wk_sb.rearrange("p (c w) -> p c w", c=CH)
        wqs = chunked(wqT_d, GD)
        wks = chunked(wkT_d, GD)
        nc.scalar.dma_start(wqv[:, 0:4], wqs[:, 0:4])
        nc.scalar.dma_start(wkv[:, 0:4], wks[:, 0:4])
        for lo, hi in ((0, 1), (1, 2), (2, 3), (3, 4), (4, 6), (6, 8)):
            nc.sync.dma_start(xv[:, lo:hi], xs[:, lo:hi])
        nc.scalar.dma_start(wqv[:, 4:8], wqs[:, 4:8])
        nc.scalar.dma_start(wkv[:, 4:8], wks[:, 4:8])
        # late-needed loads ride the SP queue AFTER x: per-queue serial DMA
        # keeps them from jumping ahead of the x chunks on the shared engines
        nc.sync.dma_start(wv_sb.rearrange("p (c w) -> p c w", c=CH),
                          chunked(wvT_d, GD))
        nc.sync.dma_start(cosT[:], cosT_d.ap())
        nc.sync.dma_start(sinTs[:], sinTs_d.ap())
        nc.sync.dma_start(bq_sb[:], bq_d.ap())
        nc.sync.dma_start(bk_sb[:], bk_d.ap())
        nc.sync.dma_start(bqs_sb[:], bqs_d.ap())
        nc.sync.dma_start(bks_sb[:], bks_d.ap())
        nc.sync.dma_start(masks[:], masks_d.ap())
        nc.scalar.dma_start(wo_sb.rearrange("p (c w) -> p c w", c=2),
                            chunked(woT_d, H))

        def xc(c):
            return x_sb[:, c * S:(c + 1) * S]

        def wc(w_sb, c, width=GD):
            return w_sb[:, c * width:(c + 1) * width]

        q_sb = [qk.tile([P, S], BF16, name=f"q{m}") for m in range(2)]
        k_sb = [qk.tile([P, S], BF16, name=f"k{m}") for m in range(2)]
        qs_sb = [qk.tile([P, S], BF16, name=f"qs{m}") for m in range(2)]
        ks_sb = [qk.tile([P, S], BF16, name=f"ks{m}") for m in range(2)]
        v_sb = [vp.tile([P, HG * VW], BF16, name=f"v{t}") for t in range(NQT)]
        ctx_sb = [cxp.tile([P, S], BF16, name=f"cx{m}") for m in range(2)]
        cx_raw = [cxp.tile([VW, 2 * S], BF16, name=f"cxr{m}") for m in range(2)]
        _osb = [osb.tile([P, H], F32, tag="osb", name=f"ot{t}", bufs=4)
                for t in range(NQT)]

        # ones columns for the softmax denominators (Pool, idle early)
        nc.gpsimd.memset(warm_in[:], 0.5)
        for t in range(NQT):
            nc.gpsimd.memset(v_sb[t][:], 1.0)

        def rope_piece(m, lo, hi, src, shf, bc, bs):
            cl = slice(lo, hi)
            for hb in range(2):
                o = hb * HD
                nc.sync.dma_start(shf[m][o:o + 32, cl],
                                  src[m][o + 32:o + 64, cl])
                nc.sync.dma_start(shf[m][o + 32:o + 64, cl],
                                  src[m][o:o + 32, cl])
            nc.vector.scalar_tensor_tensor(
                shf[m][:, cl], shf[m][:, cl], bs[:, m:m + 1], sinTs[:, cl],
                mybir.AluOpType.add, mybir.AluOpType.mult)
            nc.vector.scalar_tensor_tensor(
                src[m][:, cl], src[m][:, cl], bc[:, m:m + 1], cosT[:, cl],
                mybir.AluOpType.add, mybir.AluOpType.mult)
            nc.vector.tensor_add(src[m][:, cl], src[m][:, cl], shf[m][:, cl])

        def attn_scores(sp, mt, qi):
            kt0 = max(0, qi - WT)
            # block order: [diag, far?, middles...]
            kts = [qi]
            n_edge = 1
            if qi >= WT:
                kts.append(kt0)
                n_edge = 2
            kts.extend(range(kt0 + (1 if qi >= WT else 0), qi))

            s_ps = [sp.tile([P, WT * P + P], F32, tag="sp",
                            name=f"sps{mt}{qi}{hb}") for hb in range(2)]
            # interleave the two heads' score matmuls: lhsT base partitions
            # 0/64 give disjoint PE row groups -> concurrent on hardware
            for i, kt in enumerate(kts):
                for hb in range(2):
                    ho = hb * HD
                    nc.tensor.matmul(
                        s_ps[hb][:, i * P:(i + 1) * P],
                        k_sb[mt][ho:ho + HD, kt * P:(kt + 1) * P],
                        q_sb[mt][ho:ho + HD, qi * P:(qi + 1) * P],
                        start=True, stop=True)
            prb = []
            for hb in range(2):
                h = mt * 2 + hb
                nkt = len(kts)
                probs = pp.tile([P, WT * P + P], BF16, tag="pp",
                                name=f"pr{h}{qi}")
                nc.scalar.activation(
                    probs[:, 0:nkt * P], s_ps[hb][:, 0:nkt * P],
                    mybir.ActivationFunctionType.Exp,
                    scale=float(1.0 / np.sqrt(HD)))
                nc.vector.tensor_mul(
                    probs[:, 0:n_edge * P], probs[:, 0:n_edge * P],
                    masks[:, 0:n_edge * P])
                prb.append(probs)
            return kts, n_edge, prb

        def attn_ctx(cp, mt, qi, kts, n_edge, prb):
            nkt = len(kts)
            ctx_ps = cp.tile([VW, 2 * P], F32, tag="ctx", name=f"cps{mt}{qi}")
            for hb in range(2):
                h = mt * 2 + hb
                # ctx^T [65, q]: middles first, masked edge blocks last
                issue = list(range(n_edge, nkt)) + list(range(n_edge))
                for j, i in enumerate(issue):
                    nc.tensor.matmul(
                        ctx_ps[:, hb * P:(hb + 1) * P],
                        v_sb[kts[i]][:, h * VW:(h + 1) * VW],
                        prb[hb][:, i * P:(i + 1) * P],
                        start=(j == 0), stop=(j == nkt - 1))
            # drain PSUM with one unnormalized copy (incl. the sums row)
            nc.vector.tensor_copy(
                cx_raw[mt][:, qi * 2 * P:(qi + 1) * 2 * P], ctx_ps[:])

        def norm2(mt, qi0):
            # normalize q-tiles qi0, qi0+1 (broadcast + recip + 4 muls)
            c0 = qi0 * 2 * P
            rinv = sm.tile([1, 4 * P], F32, tag="rinv", name=f"ri{mt}{qi0}")
            nc.vector.reciprocal(rinv[:], cx_raw[mt][HD:HD + 1, c0:c0 + 4 * P])
            rbc = sm.tile([P, 4 * P], F32, tag="rbc", name=f"rb{mt}{qi0}")
            nc.gpsimd.partition_broadcast(rbc[:], rinv[:])
            for qj in (qi0, qi0 + 1):
                for hb in range(2):
                    ho = hb * HD
                    nc.gpsimd.tensor_mul(
                        ctx_sb[mt][ho:ho + HD, qj * P:(qj + 1) * P],
                        cx_raw[mt][0:HD, qj * 2 * P + hb * P:
                                   qj * 2 * P + (hb + 1) * P],
                        rbc[0:HD, (qj - qi0) * 2 * P + hb * P:
                            (qj - qi0) * 2 * P + (hb + 1) * P])

        def vchain(cp, t):
            ps = cp.tile([P, 512], F32, tag="ctx", name=f"pv{t}")
            for c in range(CH):
                nc.tensor.matmul(ps[:, 0:GD], xc(c)[:, t * P:(t + 1) * P],
                                 wc(wv_sb, c), start=(c == 0),
                                 stop=(c == CH - 1))
            vdst = v_sb[t].rearrange("p (h d) -> p h d", h=HG)[:, :, 0:HD]
            vsrc = ps[:, 0:GD].rearrange("p (h d) -> p h d", h=HG)
            nc.scalar.copy(vdst, vsrc)

        def m1half(cp, w_sb, dest, n, half):
            lo = n * 512 + half * 256
            ps = cp.tile([P, 512], F32, tag="ctx", name=f"pm1{n}{half}")
            for c in range(CH):
                nc.tensor.matmul(
                    ps[:, 0:256], wc(w_sb, c)[:, P:2 * P],
                    xc(c)[:, lo:lo + 256],
                    start=(c == 0), stop=(c == CH - 1))
            nc.vector.tensor_copy(dest[1][:, lo:lo + 256], ps[:, 0:256])

        def outproj_t(cp, t, last=False):
            o_sb = _osb[t]
            for n in range(2):
                ps = cp.tile([P, 512], F32, tag="ctx", name=f"po{t}{n}")
                for c in range(2):
                    nc.tensor.matmul(
                        ps[:], ctx_sb[c][:, t * P:(t + 1) * P],
                        wc(wo_sb, c, H)[:, n * 512:(n + 1) * 512],
                        start=(c == 0), stop=(c == 1))
                nc.vector.tensor_copy(o_sb[:, n * 512:(n + 1) * 512], ps[:])
                if last:
                    # past the final exps: halves go out on both queues
                    e = nc.sync if n == 0 else nc.scalar
                    e.dma_start(out_d.ap()[t * P:(t + 1) * P,
                                           n * 512:(n + 1) * 512],
                                o_sb[:, n * 512:(n + 1) * 512])
            if not last:
                nc.sync.dma_start(out_d.ap()[t * P:(t + 1) * P, :], o_sb[:])

        # ---- phase 1: warmup + QK m=0 projection + rope(0) ----
        with tc.tile_pool(name="pj", bufs=8, space="PSUM") as pj:
            # warmup + inter-chunk padding: one long accumulation chain so
            # the PE p-state never drops while the x stream catches up
            warm = pj.tile([P, 512], F32, tag="pj", name="warm")
            warm_n = K_WARM + 6 * 3
            wi = [0]

            def warm_mm():
                nc.tensor.matmul(warm[:, 0:2 * P], warm_in[:, 0:P],
                                 warm_in[:], start=(wi[0] == 0),
                                 stop=(wi[0] == warm_n - 1))
                wi[0] += 1

            for i in range(K_WARM):
                warm_mm()
            pss = [[pj.tile([P, 512], F32, tag="pj", name=f"pj{i}{n}")
                    for n in range(4)] for i in range(2)]
            for c in range(CH):
                for i, w in enumerate((wq_sb, wk_sb)):
                    for n in range(4):
                        nc.tensor.matmul(
                            pss[i][n][:], wc(w, c)[:, 0:P],
                            xc(c)[:, n * 512:(n + 1) * 512],
                            start=(c == 0), stop=(c == CH - 1))
                if c < 6:
                    for _ in range(3):
                        warm_mm()
            # drain + rope per quarter so attention can start after quarter 0
            for n in range(4):
                for i, d in enumerate((q_sb, k_sb)):
                    nc.scalar.copy(d[0][:, n * 512:(n + 1) * 512],
                                   pss[i][n][:])
                rope_piece(0, n * 512, (n + 1) * 512,
                           q_sb, qs_sb, bq_sb, bqs_sb)
                rope_piece(0, n * 512, (n + 1) * 512,
                           k_sb, ks_sb, bk_sb, bks_sb)

        # ---- main phase: attention A (+fillers), attention B (+outproj) ----
        with tc.tile_pool(name="sp", bufs=2, space="PSUM") as sp, \
             tc.tile_pool(name="cp", bufs=2, space="PSUM") as cp:
            # m=1 half-chain order feeds rope(1) quarter by quarter
            m1args = [(wq_sb, q_sb, 0), (wq_sb, q_sb, 1),
                      (wk_sb, k_sb, 0), (wk_sb, k_sb, 1),
                      (wq_sb, q_sb, 2), (wq_sb, q_sb, 3),
                      (wk_sb, k_sb, 2), (wk_sb, k_sb, 3)]

            def m1_step(j):
                w_sb_, dest_, n_ = m1args[j]
                m1half(cp, w_sb_, dest_, n_, 0)
                m1half(cp, w_sb_, dest_, n_, 1)
                shf_ = qs_sb if dest_ is q_sb else ks_sb
                bc_ = bq_sb if dest_ is q_sb else bk_sb
                bs_ = bqs_sb if dest_ is q_sb else bks_sb
                rope_piece(1, n_ * 512, (n_ + 1) * 512,
                           dest_, shf_, bc_, bs_)

            # fillers before pass A: V tiles 0-3, first m=1 quarter
            for t in range(4):
                vchain(cp, t)
            m1_step(0)

            # ---- pass A (mt=0), ascending ----
            pend = None
            for qi in range(NQT):
                cur = attn_scores(sp, 0, qi)
                if pend is not None:
                    attn_ctx(cp, 0, qi - 1, *pend)
                    if qi % 2 == 0 and qi >= 2:
                        norm2(0, qi - 2)
                pend = cur
                # PE filler work, after the critical attention ops
                if qi + 4 < NQT:
                    vchain(cp, qi + 4)
                if qi <= 6:
                    m1_step(qi + 1)
            attn_ctx(cp, 0, NQT - 1, *pend)
            norm2(0, NQT - 2)

            # ---- pass B (mt=1), descending; outproj trails 3 ----
            pend = None
            for qi in range(NQT - 1, -1, -1):
                cur = attn_scores(sp, 1, qi)
                if pend is not None:
                    attn_ctx(cp, 1, qi + 1, *pend)
                    if qi % 2 == 1:
                        norm2(1, qi + 1)
                if qi <= 12:
                    outproj_t(cp, qi + 3)
                pend = cur
            attn_ctx(cp, 1, 0, *pend)
            norm2(1, 0)
            outproj_t(cp, 2)
            outproj_t(cp, 1, last=True)
            outproj_t(cp, 0, last=True)

    nc.compile()
    return nc


def _rope_tables():
    inv_freq = 1.0 / (ROPE_THETA ** (np.arange(0, HD, 2, dtype=np.float64) / HD))
    t = np.arange(MAX_POS, dtype=np.float64)
    freqs = np.outer(t, inv_freq)                       # [MAX_POS, 32]
    emb = np.concatenate([freqs, freqs], axis=-1)       # [MAX_POS, 64]
    return np.cos(emb).astype(np.float32), np.sin(emb).astype(np.float32)


def kernel(hidden_states, position_ids, wq, bq, wk, bk, wv, bv, wo, bo):
    bf16 = ml_dtypes.bfloat16
    if "nc" not in _cache:
        _cache["nc"] = _build()
    nc = _cache["nc"]

    cos_t, sin_t = _rope_tables()
    pos = np.clip(np.asarray(position_ids), 0, MAX_POS - 1).astype(np.int64)

    maskd = np.triu(np.ones((P, P), np.float32))        # k <= q (diag block)
    maskf = np.tril(np.ones((P, P), np.float32), -1)    # k > q  (far block)
    masks = np.concatenate([maskd, maskf], axis=1).astype(bf16)

    in_maps = []
    for core in range(N_CORES):
        b, g = core // HG, core % HG
        sl = slice(g * GD, (g + 1) * GD)
        cos_b = cos_t[pos[b]]                            # [S, 64]
        sin_b = sin_t[pos[b]]
        cosT = np.tile(cos_b.T, (2, 1)).astype(bf16)     # [128, S]
        sin_sgn = sin_b.T.copy()                         # [64, S]
        sin_sgn[0:32] *= -1.0
        sinTs = np.tile(sin_sgn, (2, 1)).astype(bf16)
        in_maps.append({
            "xT": np.ascontiguousarray(hidden_states[b].T).astype(bf16),
            "wqT": np.ascontiguousarray(wq[sl].T).astype(bf16),
            "wkT": np.ascontiguousarray(wk[sl].T).astype(bf16),
            "wvT": np.ascontiguousarray(wv[sl].T).astype(bf16),
            "woT": np.ascontiguousarray(wo[:, sl].T).astype(bf16),
            "cosT": cosT,
            "sinTs": sinTs,
            "bq2": np.ascontiguousarray(
                bq[sl].reshape(2, P).T).astype(np.float32),
            "bk2": np.ascontiguousarray(
                bk[sl].reshape(2, P).T).astype(np.float32),
            "bq2s": np.ascontiguousarray(
                bq[sl].reshape(2, 2, 2, 32)[:, :, ::-1].reshape(
                    2, P).T).astype(np.float32),
            "bk2s": np.ascontiguousarray(
                bk[sl].reshape(2, 2, 2, 32)[:, :, ::-1].reshape(
                    2, P).T).astype(np.float32),
            "masks": masks,
        })

    res = run_bass_kernel_spmd(nc, in_maps, core_ids=list(range(N_CORES)))

    const_off = (wo @ bv + bo).astype(np.float32)        # host-folded biases
    out = np.empty((B, S, H), dtype=np.float32)
    for b in range(B):
        acc = res.results[b * HG]["out"].astype(np.float32).copy()
        for g in range(1, HG):
            acc += res.results[b * HG + g]["out"]
        out[b] = acc + const_off[None, :]
    return out
